# revision 2
# baseline (speedup 1.0000x reference)
"""Trainium2 Bass kernel for nn_DaVinciMLP (3-modality MoE MLP).

Reference computation (per token t with modality e = modality_ids[t]):
    xn  = bf16( x * rsqrt(mean(x^2) + 1e-6) * (norm_w[e] + 1) )
    up  = xn @ up_w[e].T            # [H] -> [I]
    g   = min(up, 7) * sigmoid(1.702 * min(up, 7))
    out = g @ down_w[e].T           # [I] -> [H]

Strategy:
  - Host: sort tokens by modality id so each expert's tokens are a dense,
    contiguous (128-padded) range -> dense per-expert GEMMs instead of the
    reference's 3x-masked-dense compute.  Fold (norm_w[e] + 1) into the up
    weights.
  - Sharding: Megatron tensor-parallel on the intermediate dim I across 8
    cores (up_w sharded on out dim, down_w on in dim).  Every core sees all
    tokens and produces a partial [H, L] output; host sums partials in f32.
  - Device: transposed activations [H, tok] are loaded straight from HBM
    with XBAR DMA-transpose; per 128-token tile the sum-of-squares runs on
    ScalarE (fused accum), rms = 1/sqrt(mean+eps) on VectorE, and the rms
    column is turned into a row with a tiny TensorE matmul, then broadcast
    across partitions by doubling SBUF-to-SBUF DMAs.  Up GEMM accumulates
    over H in PSUM; the rms scale is applied at the gelu stage
    (min(psum*rms,7)*sigmoid(1.702*...) on Vector+Scalar); down GEMM
    accumulates over the I-shard in PSUM and streams the partial output
    back transposed ([H, L]).
"""

import os
from contextlib import ExitStack

import numpy as np
import ml_dtypes

import concourse.bass as bass
import concourse.tile as tile
from concourse import bacc, mybir
from concourse.bass_utils import run_bass_kernel_spmd

BF16 = mybir.dt.bfloat16
F32 = mybir.dt.float32
NP_BF16 = ml_dtypes.bfloat16
AF = mybir.ActivationFunctionType

N_CORES = 8
H = 5120
I_FULL = 20480
E = 3
EPS = 1e-6
P = 128
TB = 1024  # max token block resident in SBUF
CHUNK = 512  # matmul moving free dim / PSUM bank width

LAST_EXEC_NS = None  # set when BASS_TRACE=1


def _build_program(blocks, L, h, i_shard, n_exp):
    """One SPMD program for all cores; per-core data differs only in values."""
    n_ko = h // P  # k-tiles over H for up GEMM; also # of H output blocks
    n_ic = i_shard // P  # I blocks per expert shard; k-tiles for down GEMM

    nc = bacc.Bacc()
    x_ext = nc.declare_dram_parameter("x", [L, h], BF16, isOutput=False)
    wup_ext = nc.declare_dram_parameter(
        "wup", [n_exp, n_ic, P, n_ko, P], BF16, isOutput=False
    )
    wd_ext = nc.declare_dram_parameter(
        "wd", [n_exp, n_ko, P, n_ic, P], BF16, isOutput=False
    )
    out_ext = nc.declare_dram_parameter("out", [h, L], BF16, isOutput=True)

    with tile.TileContext(nc) as tc, ExitStack() as ctx:
        const_pool = ctx.enter_context(tc.tile_pool(name="const", bufs=1))
        x_pool = ctx.enter_context(tc.tile_pool(name="x", bufs=2))
        sq_pool = ctx.enter_context(tc.tile_pool(name="sq", bufs=1))
        small_pool = ctx.enter_context(tc.tile_pool(name="small", bufs=4))
        rbc_pool = ctx.enter_context(tc.tile_pool(name="rbc", bufs=2))
        rrow_pool = ctx.enter_context(tc.tile_pool(name="rrow", bufs=1))
        xT_pool = ctx.enter_context(tc.tile_pool(name="xT", bufs=1))
        g_pool = ctx.enter_context(tc.tile_pool(name="g", bufs=1))
        wu_pool = ctx.enter_context(tc.tile_pool(name="wu", bufs=3))
        wd_pool = ctx.enter_context(tc.tile_pool(name="wd", bufs=3))
        act_pool = ctx.enter_context(tc.tile_pool(name="act", bufs=2))
        ob_pool = ctx.enter_context(tc.tile_pool(name="ob", bufs=4))
        row_psum = ctx.enter_context(tc.tile_pool(name="rowps", bufs=1, space="PSUM"))
        up_psum = ctx.enter_context(tc.tile_pool(name="upps", bufs=4, space="PSUM"))
        dn_psum = ctx.enter_context(tc.tile_pool(name="dnps", bufs=2, space="PSUM"))

        from concourse.masks import make_identity

        ident_f = const_pool.tile([P, P], F32)
        make_identity(nc, ident_f)

        for (e, t0, ntok) in blocks:
            nt = (ntok + P - 1) // P
            xT = xT_pool.tile([P, n_ko, TB], BF16, tag="xT")
            gt = g_pool.tile([P, n_ic, TB], BF16, tag="g")
            rms_bc = rbc_pool.tile([P, TB], BF16, tag="rbc")
            rr_ps = row_psum.tile([1, TB], F32, tag="rowps")

            # ---- transposed activation load (pure DMA via XBAR)
            for ko in range(n_ko):
                nc.sync.dma_start_transpose(
                    xT[:, ko, :ntok], x_ext[t0 : t0 + ntok, ko * P : (ko + 1) * P]
                )

            # prefetch the first up-weight blocks ahead of the x-stat loads
            wu_pref = {}
            for ic in range(min(2, n_ic)):
                wu = wu_pool.tile([P, n_ko, P], BF16, tag="wu")
                nc.sync.dma_start(out=wu[:], in_=wup_ext[e, ic])
                wu_pref[ic] = wu

            # ---- per-token rms: ssq on ScalarE, then row-ify via PE
            for t in range(nt):
                rt = min(P, ntok - t * P)
                xtile = x_pool.tile([P, h], BF16, tag="x")
                nc.sync.dma_start(
                    out=xtile[:rt], in_=x_ext[t0 + t * P : t0 + t * P + rt, :]
                )
                h2 = h // 2
                ssq2 = small_pool.tile([P, 2], F32, tag="ssq2")
                for half in range(2):
                    sq = sq_pool.tile([P, h2], BF16, tag="sq")
                    nc.scalar.activation(
                        sq[:rt],
                        xtile[:rt, half * h2 : (half + 1) * h2],
                        AF.Square,
                        accum_out=ssq2[:rt, half : half + 1],
                    )
                ssq = small_pool.tile([P, 1], F32, tag="ssq")
                nc.vector.tensor_tensor(
                    ssq[:rt], ssq2[:rt, 0:1], ssq2[:rt, 1:2], mybir.AluOpType.add
                )
                mn = small_pool.tile([P, 1], F32, tag="mn")
                nc.vector.tensor_scalar(
                    mn[:rt], ssq[:rt], 1.0 / h, EPS, mybir.AluOpType.mult, mybir.AluOpType.add
                )
                s_ = small_pool.tile([P, 1], F32, tag="s")
                nc.scalar.activation(s_[:rt], mn[:rt], AF.Sqrt)
                rms = small_pool.tile([P, 1], F32, tag="rms")
                nc.vector.reciprocal(rms[:rt], s_[:rt])
                # rr_ps[0, t*P + j] = rms[j]
                nc.tensor.matmul(
                    rr_ps[0:1, t * P : t * P + rt],
                    lhsT=rms[:rt, 0:1],
                    rhs=ident_f[:rt, :rt],
                    start=True,
                    stop=True,
                )
            # row -> bf16, then log-broadcast down the partitions via DMA
            rrow = rrow_pool.tile([1, TB], BF16, tag="rrow")
            nc.vector.tensor_copy(out=rrow[0:1, :ntok], in_=rr_ps[0:1, :ntok])
            nc.sync.dma_start(out=rms_bc[0:1, :ntok], in_=rrow[0:1, :ntok])
            k = 1
            while k < P:
                kk = min(k, P - k)
                nc.sync.dma_start(
                    out=rms_bc[k : k + kk, :ntok], in_=rms_bc[0:kk, :ntok]
                )
                k += kk

            chunks = []
            c0 = 0
            while c0 < ntok:
                cw = min(CHUNK, ntok - c0)
                chunks.append((c0, cw))
                c0 += cw

            # ---- up GEMM + rms scale + gelu7 -> gt
            for ic in range(n_ic):
                if ic in wu_pref:
                    wu = wu_pref.pop(ic)
                else:
                    wu = wu_pool.tile([P, n_ko, P], BF16, tag="wu")
                    nc.sync.dma_start(out=wu[:], in_=wup_ext[e, ic])
                for (c0, cw) in chunks:
                    ups = up_psum.tile([P, CHUNK], F32, tag="upps")
                    for ko in range(n_ko):
                        nc.tensor.matmul(
                            ups[:, :cw],
                            lhsT=wu[:, ko, :],
                            rhs=xT[:, ko, c0 : c0 + cw],
                            start=(ko == 0),
                            stop=(ko == n_ko - 1),
                        )
                    tmin = act_pool.tile([P, CHUNK], BF16, tag="tmin")
                    nc.vector.tensor_tensor(
                        tmin[:, :cw],
                        ups[:, :cw],
                        rms_bc[:, c0 : c0 + cw],
                        mybir.AluOpType.mult,
                    )
                    nc.vector.tensor_scalar_min(tmin[:, :cw], tmin[:, :cw], 7.0)
                    sgm = act_pool.tile([P, CHUNK], BF16, tag="sgm")
                    nc.scalar.activation(sgm[:, :cw], tmin[:, :cw], AF.Sigmoid, scale=1.702)
                    nc.vector.tensor_mul(
                        out=gt[:, ic, c0 : c0 + cw], in0=tmin[:, :cw], in1=sgm[:, :cw]
                    )

            # ---- down GEMM -> partial out (transposed [H, L])
            for hc in range(n_ko):
                wdt = wd_pool.tile([P, n_ic, P], BF16, tag="wd")
                nc.sync.dma_start(out=wdt[:], in_=wd_ext[e, hc])
                for (c0, cw) in chunks:
                    dps = dn_psum.tile([P, CHUNK], F32, tag="dnps")
                    for ko in range(n_ic):
                        nc.tensor.matmul(
                            dps[:, :cw],
                            lhsT=wdt[:, ko, :],
                            rhs=gt[:, ko, c0 : c0 + cw],
                            start=(ko == 0),
                            stop=(ko == n_ic - 1),
                        )
                    ob = ob_pool.tile([P, CHUNK], BF16, tag="ob")
                    nc.vector.tensor_copy(out=ob[:, :cw], in_=dps[:, :cw])
                    nc.sync.dma_start(
                        out=out_ext[hc * P : (hc + 1) * P, t0 + c0 : t0 + c0 + cw],
                        in_=ob[:, :cw],
                    )
    nc.compile()
    return nc


def _plan_blocks(ids, n_exp):
    """Sort tokens by expert, pad each segment to a multiple of 16 (XBAR row
    granularity), split into blocks of <= TB tokens (one expert per block)."""
    idx = [np.nonzero(ids == e)[0] for e in range(n_exp)]
    segs = []  # (expert, seg_start, n_valid)
    blocks = []  # (expert, tok_start, n_tok_padded)
    t0 = 0
    for e in range(n_exp):
        c = len(idx[e])
        if c == 0:
            continue
        cpad = ((c + 15) // 16) * 16
        off = 0
        while off < cpad:
            nb = min(TB, cpad - off)
            blocks.append((e, t0 + off, nb))
            off += nb
        segs.append((e, t0, c))
        t0 += cpad
    return idx, segs, blocks, t0


def _prep_weights(up_w, down_w, norm_w, h, i_full, n_exp, n_cores):
    """Fold (norm_w+1) into up weights; build per-core contiguous block
    layouts: wup [E, n_ic, ki, ko, m] (ki over H, m over I) and
    wd [E, n_hc, ki, ko, m] (ki over I, m over H)."""
    i_shard = i_full // n_cores
    n_ic = i_shard // P

    up = up_w.reshape(n_exp, i_full, h)
    dn = down_w.reshape(n_exp, h, i_full)
    w1 = norm_w.reshape(n_exp, 1, h).astype(np.float32) + 1.0

    # A[e, icg, ki, ko, m] = up[e, icg*P+m, ko*P+ki] * (norm_w[e, ko*P+ki]+1)
    A = np.empty((n_exp, i_full // P, P, h // P, P), dtype=NP_BF16)
    for e in range(n_exp):
        Ae = (up[e].astype(np.float32) * w1[e]).astype(NP_BF16)  # [I, H]
        A[e] = Ae.reshape(i_full // P, P, h // P, P).transpose(0, 3, 2, 1)
    # Bf[e, hc, ki, kog, m] = dn[e, hc*P+m, kog*P+ki]
    Bf = np.empty((n_exp, h // P, P, i_full // P, P), dtype=NP_BF16)
    for e in range(n_exp):
        Be = dn[e].astype(NP_BF16)  # [H, I]
        Bf[e] = Be.reshape(h // P, P, i_full // P, P).transpose(0, 3, 2, 1)

    wups, wds = [], []
    for c in range(n_cores):
        wups.append(np.ascontiguousarray(A[:, c * n_ic : (c + 1) * n_ic]))
        wds.append(np.ascontiguousarray(Bf[:, :, :, c * n_ic : (c + 1) * n_ic, :]))
    return wups, wds


def _prepare(inputs):
    """Host prep: sort tokens, fold norm into up weights, build the program.
    Returns (nc, in_maps, ctx) where ctx carries what _finish needs."""
    # NTFF tracing needs axon hooks that aren't present in the sandbox; make
    # sure a stray BASS_TRACE can't divert run_bass_kernel_spmd into it.
    os.environ["BASS_NEVER_TRACE"] = "1"
    x = np.asarray(inputs["x"])
    ids = np.asarray(inputs["modality_ids"]).astype(np.int64)
    norm_w = np.asarray(inputs["norm_w"])
    up_w = np.asarray(inputs["up_w"])
    down_w = np.asarray(inputs["down_w"])

    n_tok, h = x.shape
    i_full = up_w.shape[0] // E
    assert down_w.shape == (E * h, i_full)
    if x.dtype != NP_BF16:
        x = x.astype(NP_BF16)

    idx, segs, blocks, L = _plan_blocks(ids, E)
    x_sorted = np.zeros((L, h), dtype=NP_BF16)
    for (e, s0, c) in segs:
        x_sorted[s0 : s0 + c] = x[idx[e]]

    wups, wds = _prep_weights(up_w, down_w, norm_w, h, i_full, E, N_CORES)

    nc = _build_program(blocks, L, h, i_full // N_CORES, E)
    in_maps = [{"x": x_sorted, "wup": wups[c], "wd": wds[c]} for c in range(N_CORES)]
    ctx = dict(idx=idx, segs=segs, L=L, h=h, n_tok=n_tok)
    return nc, in_maps, ctx


def _finish(results, ctx):
    """Sum per-core partials ([H, L] each), unsort, cast to bf16."""
    h, L, n_tok = ctx["h"], ctx["L"], ctx["n_tok"]
    acc = np.zeros((h, L), dtype=np.float32)
    for r in results:
        acc += np.asarray(r["out"], dtype=np.float32)
    out_sorted = acc.T  # [L, h]
    out = np.empty((n_tok, h), dtype=np.float32)
    for (e, s0, c) in ctx["segs"]:
        out[ctx["idx"][e]] = out_sorted[s0 : s0 + c]
    return out.astype(NP_BF16)


def kernel(**inputs):
    global LAST_EXEC_NS
    nc, in_maps, ctx = _prepare(inputs)
    res = run_bass_kernel_spmd(nc, in_maps, core_ids=list(range(N_CORES)))
    LAST_EXEC_NS = res.exec_time_ns
    return _finish(res.results, ctx)



# revision 5
# speedup vs baseline: 10.0531x; 10.0531x over previous
"""Trainium2 Bass kernel for nn_DaVinciMLP (3-modality MoE MLP).

Reference computation (per token t with modality e = modality_ids[t]):
    xn  = bf16( x * rsqrt(mean(x^2) + 1e-6) * (norm_w[e] + 1) )
    up  = xn @ up_w[e].T            # [H] -> [I]
    g   = min(up, 7) * sigmoid(1.702 * min(up, 7))
    out = g @ down_w[e].T           # [I] -> [H]

Strategy:
  - Host: sort tokens by modality id so each expert's tokens are a dense,
    contiguous (128-padded) range -> dense per-expert GEMMs instead of the
    reference's 3x-masked-dense compute.  Fold (norm_w[e] + 1) into the up
    weights.
  - Sharding: Megatron tensor-parallel on the intermediate dim I across 8
    cores (up_w sharded on out dim, down_w on in dim).  Every core sees all
    tokens and produces a partial [H, L] output; host sums partials in f32.
  - Device: transposed activations [H, tok] are loaded straight from HBM
    with XBAR DMA-transpose; per 128-token tile the sum-of-squares runs on
    ScalarE (fused accum), rms = 1/sqrt(mean+eps) on VectorE, and the rms
    column is turned into a row with a tiny TensorE matmul, then broadcast
    across partitions by doubling SBUF-to-SBUF DMAs.  Up GEMM accumulates
    over H in PSUM; the rms scale is applied at the gelu stage
    (min(psum*rms,7)*sigmoid(1.702*...) on Vector+Scalar); down GEMM
    accumulates over the I-shard in PSUM and streams the partial output
    back transposed ([H, L]).
"""

import os
from contextlib import ExitStack

import numpy as np
import ml_dtypes

import concourse.bass as bass
import concourse.tile as tile
from concourse import bacc, mybir
from concourse.bass_utils import run_bass_kernel_spmd

BF16 = mybir.dt.bfloat16
F32 = mybir.dt.float32
NP_BF16 = ml_dtypes.bfloat16
AF = mybir.ActivationFunctionType

N_CORES = 8
H = 5120
I_FULL = 20480
E = 3
EPS = 1e-6
P = 128
TB = 1024  # max token block resident in SBUF
CHUNK = 512  # matmul moving free dim / PSUM bank width

LAST_EXEC_NS = None  # set when BASS_TRACE=1


def _build_program(blocks, L, h, i_shard, n_exp, reps=1):
    """One SPMD program for all cores; per-core data differs only in values.

    reps > 1 wraps the whole body in a hardware loop that recomputes the
    identical output `reps` times — used only by bench.py to separate device
    time from tunnel/dispatch overhead ((wall(R) - wall(1)) / (R - 1))."""
    n_ko = h // P  # k-tiles over H for up GEMM; also # of H output blocks
    n_ic = i_shard // P  # I blocks per expert shard; k-tiles for down GEMM

    nc = bacc.Bacc()
    x_ext = nc.declare_dram_parameter("x", [L, h], BF16, isOutput=False)
    wup_ext = nc.declare_dram_parameter(
        "wup", [n_exp, n_ic, P, n_ko, P], BF16, isOutput=False
    )
    wd_ext = nc.declare_dram_parameter(
        "wd", [n_exp, n_ko, P, n_ic, P], BF16, isOutput=False
    )
    out_ext = nc.declare_dram_parameter("out", [h, L], BF16, isOutput=True)

    with tile.TileContext(nc) as tc, ExitStack() as ctx:
        const_pool = ctx.enter_context(tc.tile_pool(name="const", bufs=1))
        x_pool = ctx.enter_context(tc.tile_pool(name="x", bufs=2))
        sq_pool = ctx.enter_context(tc.tile_pool(name="sq", bufs=1))
        small_pool = ctx.enter_context(tc.tile_pool(name="small", bufs=4))
        rbc_pool = ctx.enter_context(tc.tile_pool(name="rbc", bufs=2))
        rrow_pool = ctx.enter_context(tc.tile_pool(name="rrow", bufs=1))
        xT_pool = ctx.enter_context(tc.tile_pool(name="xT", bufs=1))
        g_pool = ctx.enter_context(tc.tile_pool(name="g", bufs=1))
        wu_pool = ctx.enter_context(tc.tile_pool(name="wu", bufs=3))
        wd_pool = ctx.enter_context(tc.tile_pool(name="wd", bufs=3))
        act_pool = ctx.enter_context(tc.tile_pool(name="act", bufs=2))
        ob_pool = ctx.enter_context(tc.tile_pool(name="ob", bufs=4))
        row_psum = ctx.enter_context(tc.tile_pool(name="rowps", bufs=1, space="PSUM"))
        up_psum = ctx.enter_context(tc.tile_pool(name="upps", bufs=4, space="PSUM"))
        dn_psum = ctx.enter_context(tc.tile_pool(name="dnps", bufs=2, space="PSUM"))

        from concourse.masks import make_identity

        ident_f = const_pool.tile([P, P], F32)
        make_identity(nc, ident_f)

        rep_loop = tc.For_i(0, reps) if reps > 1 else None
        if rep_loop is not None:
            rep_loop.__enter__()

        for (e, t0, ntok) in blocks:
            nt = (ntok + P - 1) // P
            xT = xT_pool.tile([P, n_ko, TB], BF16, tag="xT")
            gt = g_pool.tile([P, n_ic, TB], BF16, tag="g")
            rms_bc = rbc_pool.tile([P, TB], BF16, tag="rbc")
            rr_ps = row_psum.tile([1, TB], F32, tag="rowps")

            # ---- transposed activation load (pure DMA via XBAR)
            for ko in range(n_ko):
                nc.sync.dma_start_transpose(
                    xT[:, ko, :ntok], x_ext[t0 : t0 + ntok, ko * P : (ko + 1) * P]
                )

            # prefetch the first up-weight blocks ahead of the x-stat loads
            wu_pref = {}
            for ic in range(min(2, n_ic)):
                wu = wu_pool.tile([P, n_ko, P], BF16, tag="wu")
                nc.sync.dma_start(out=wu[:], in_=wup_ext[e, ic])
                wu_pref[ic] = wu

            # ---- per-token rms: ssq on ScalarE, then row-ify via PE
            for t in range(nt):
                rt = min(P, ntok - t * P)
                xtile = x_pool.tile([P, h], BF16, tag="x")
                nc.sync.dma_start(
                    out=xtile[:rt], in_=x_ext[t0 + t * P : t0 + t * P + rt, :]
                )
                h2 = h // 2
                ssq2 = small_pool.tile([P, 2], F32, tag="ssq2")
                for half in range(2):
                    sq = sq_pool.tile([P, h2], BF16, tag="sq")
                    nc.scalar.activation(
                        sq[:rt],
                        xtile[:rt, half * h2 : (half + 1) * h2],
                        AF.Square,
                        accum_out=ssq2[:rt, half : half + 1],
                    )
                ssq = small_pool.tile([P, 1], F32, tag="ssq")
                nc.vector.tensor_tensor(
                    ssq[:rt], ssq2[:rt, 0:1], ssq2[:rt, 1:2], mybir.AluOpType.add
                )
                mn = small_pool.tile([P, 1], F32, tag="mn")
                nc.vector.tensor_scalar(
                    mn[:rt], ssq[:rt], 1.0 / h, EPS, mybir.AluOpType.mult, mybir.AluOpType.add
                )
                s_ = small_pool.tile([P, 1], F32, tag="s")
                nc.scalar.activation(s_[:rt], mn[:rt], AF.Sqrt)
                rms = small_pool.tile([P, 1], F32, tag="rms")
                nc.vector.reciprocal(rms[:rt], s_[:rt])
                # rr_ps[0, t*P + j] = rms[j]
                nc.tensor.matmul(
                    rr_ps[0:1, t * P : t * P + rt],
                    lhsT=rms[:rt, 0:1],
                    rhs=ident_f[:rt, :rt],
                    start=True,
                    stop=True,
                )
            # row -> bf16, then log-broadcast down the partitions via DMA
            rrow = rrow_pool.tile([1, TB], BF16, tag="rrow")
            nc.vector.tensor_copy(out=rrow[0:1, :ntok], in_=rr_ps[0:1, :ntok])
            nc.sync.dma_start(out=rms_bc[0:1, :ntok], in_=rrow[0:1, :ntok])
            k = 1
            while k < P:
                kk = min(k, P - k)
                nc.sync.dma_start(
                    out=rms_bc[k : k + kk, :ntok], in_=rms_bc[0:kk, :ntok]
                )
                k += kk

            chunks = []
            c0 = 0
            while c0 < ntok:
                cw = min(CHUNK, ntok - c0)
                chunks.append((c0, cw))
                c0 += cw

            # ---- up GEMM + rms scale + gelu7 -> gt
            for ic in range(n_ic):
                if ic in wu_pref:
                    wu = wu_pref.pop(ic)
                else:
                    wu = wu_pool.tile([P, n_ko, P], BF16, tag="wu")
                    nc.sync.dma_start(out=wu[:], in_=wup_ext[e, ic])
                for (c0, cw) in chunks:
                    ups = up_psum.tile([P, CHUNK], F32, tag="upps")
                    for ko in range(n_ko):
                        nc.tensor.matmul(
                            ups[:, :cw],
                            lhsT=wu[:, ko, :],
                            rhs=xT[:, ko, c0 : c0 + cw],
                            start=(ko == 0),
                            stop=(ko == n_ko - 1),
                        )
                    tmin = act_pool.tile([P, CHUNK], BF16, tag="tmin")
                    nc.vector.tensor_tensor(
                        tmin[:, :cw],
                        ups[:, :cw],
                        rms_bc[:, c0 : c0 + cw],
                        mybir.AluOpType.mult,
                    )
                    nc.vector.tensor_scalar_min(tmin[:, :cw], tmin[:, :cw], 7.0)
                    sgm = act_pool.tile([P, CHUNK], BF16, tag="sgm")
                    nc.scalar.activation(sgm[:, :cw], tmin[:, :cw], AF.Sigmoid, scale=1.702)
                    nc.vector.tensor_mul(
                        out=gt[:, ic, c0 : c0 + cw], in0=tmin[:, :cw], in1=sgm[:, :cw]
                    )

            # ---- down GEMM -> partial out (transposed [H, L])
            for hc in range(n_ko):
                wdt = wd_pool.tile([P, n_ic, P], BF16, tag="wd")
                nc.sync.dma_start(out=wdt[:], in_=wd_ext[e, hc])
                for (c0, cw) in chunks:
                    dps = dn_psum.tile([P, CHUNK], F32, tag="dnps")
                    for ko in range(n_ic):
                        nc.tensor.matmul(
                            dps[:, :cw],
                            lhsT=wdt[:, ko, :],
                            rhs=gt[:, ko, c0 : c0 + cw],
                            start=(ko == 0),
                            stop=(ko == n_ic - 1),
                        )
                    ob = ob_pool.tile([P, CHUNK], BF16, tag="ob")
                    nc.vector.tensor_copy(out=ob[:, :cw], in_=dps[:, :cw])
                    nc.sync.dma_start(
                        out=out_ext[hc * P : (hc + 1) * P, t0 + c0 : t0 + c0 + cw],
                        in_=ob[:, :cw],
                    )
        if rep_loop is not None:
            rep_loop.__exit__(None, None, None)
    nc.compile()
    return nc


def _plan_blocks(ids, n_exp):
    """Sort tokens by expert, pad each segment to a multiple of 16 (XBAR row
    granularity), split into blocks of <= TB tokens (one expert per block)."""
    idx = [np.nonzero(ids == e)[0] for e in range(n_exp)]
    segs = []  # (expert, seg_start, n_valid)
    blocks = []  # (expert, tok_start, n_tok_padded)
    t0 = 0
    for e in range(n_exp):
        c = len(idx[e])
        if c == 0:
            continue
        cpad = ((c + 15) // 16) * 16
        off = 0
        while off < cpad:
            nb = min(TB, cpad - off)
            blocks.append((e, t0 + off, nb))
            off += nb
        segs.append((e, t0, c))
        t0 += cpad
    return idx, segs, blocks, t0


def _prep_weights(up_w, down_w, norm_w, h, i_full, n_exp, n_cores):
    """Fold (norm_w+1) into up weights; build per-core contiguous block
    layouts: wup [E, n_ic, ki, ko, m] (ki over H, m over I) and
    wd [E, n_hc, ki, ko, m] (ki over I, m over H)."""
    i_shard = i_full // n_cores
    n_ic = i_shard // P

    up = up_w.reshape(n_exp, i_full, h)
    dn = down_w.reshape(n_exp, h, i_full)
    w1 = norm_w.reshape(n_exp, 1, h).astype(np.float32) + 1.0

    # A[e, icg, ki, ko, m] = up[e, icg*P+m, ko*P+ki] * (norm_w[e, ko*P+ki]+1)
    A = np.empty((n_exp, i_full // P, P, h // P, P), dtype=NP_BF16)
    for e in range(n_exp):
        Ae = (up[e].astype(np.float32) * w1[e]).astype(NP_BF16)  # [I, H]
        A[e] = Ae.reshape(i_full // P, P, h // P, P).transpose(0, 3, 2, 1)
    # Bf[e, hc, ki, kog, m] = dn[e, hc*P+m, kog*P+ki]
    Bf = np.empty((n_exp, h // P, P, i_full // P, P), dtype=NP_BF16)
    for e in range(n_exp):
        Be = dn[e].astype(NP_BF16)  # [H, I]
        Bf[e] = Be.reshape(h // P, P, i_full // P, P).transpose(0, 3, 2, 1)

    wups, wds = [], []
    for c in range(n_cores):
        wups.append(np.ascontiguousarray(A[:, c * n_ic : (c + 1) * n_ic]))
        wds.append(np.ascontiguousarray(Bf[:, :, :, c * n_ic : (c + 1) * n_ic, :]))
    return wups, wds


def _prepare(inputs):
    """Host prep: sort tokens, fold norm into up weights, build the program.
    Returns (nc, in_maps, ctx) where ctx carries what _finish needs."""
    # NTFF tracing needs axon hooks that aren't present in the sandbox; make
    # sure a stray BASS_TRACE can't divert run_bass_kernel_spmd into it.
    os.environ["BASS_NEVER_TRACE"] = "1"
    x = np.asarray(inputs["x"])
    ids = np.asarray(inputs["modality_ids"]).astype(np.int64)
    norm_w = np.asarray(inputs["norm_w"])
    up_w = np.asarray(inputs["up_w"])
    down_w = np.asarray(inputs["down_w"])

    n_tok, h = x.shape
    i_full = up_w.shape[0] // E
    assert down_w.shape == (E * h, i_full)
    if x.dtype != NP_BF16:
        x = x.astype(NP_BF16)

    idx, segs, blocks, L = _plan_blocks(ids, E)
    x_sorted = np.zeros((L, h), dtype=NP_BF16)
    for (e, s0, c) in segs:
        x_sorted[s0 : s0 + c] = x[idx[e]]

    wups, wds = _prep_weights(up_w, down_w, norm_w, h, i_full, E, N_CORES)

    nc = _build_program(blocks, L, h, i_full // N_CORES, E)
    in_maps = [{"x": x_sorted, "wup": wups[c], "wd": wds[c]} for c in range(N_CORES)]
    ctx = dict(idx=idx, segs=segs, L=L, h=h, n_tok=n_tok)
    return nc, in_maps, ctx


def _finish(results, ctx):
    """Sum per-core partials ([H, L] each), unsort, cast to bf16."""
    h, L, n_tok = ctx["h"], ctx["L"], ctx["n_tok"]
    acc = np.zeros((h, L), dtype=np.float32)
    for r in results:
        acc += np.asarray(r["out"], dtype=np.float32)
    out_sorted = acc.T  # [L, h]
    out = np.empty((n_tok, h), dtype=np.float32)
    for (e, s0, c) in ctx["segs"]:
        out[ctx["idx"][e]] = out_sorted[s0 : s0 + c]
    return out.astype(NP_BF16)


def kernel(**inputs):
    global LAST_EXEC_NS
    nc, in_maps, ctx = _prepare(inputs)
    res = run_bass_kernel_spmd(nc, in_maps, core_ids=list(range(N_CORES)))
    LAST_EXEC_NS = res.exec_time_ns
    return _finish(res.results, ctx)

